# revision 15
# baseline (speedup 1.0000x reference)
"""Trainium2 kernel for nn_DWT_Features.

The reference applies a 3-level db4 DWT along the time axis of every
(batch, pixel) signal, then contracts the coefficients with a full-volume
conv kernel and applies LeakyReLU.  The DWT is a linear map sig[64] ->
coeffs[84], so the whole network collapses to a single GEMM:

    out = leaky_relu(X @ W_eff + b),  X: [B, 4096], W_eff: [4096, 64]

where W_eff[(t,h,w), k] = sum_c M[t, c] * conv_w[k, c, h, w] and M is the
64x84 DWT matrix (computed here in numpy, folded on host - O(22M) flops).

Sharding: pure data parallel, batch split across 8 cores (1024 rows each).

Host prep (sharding/relayout): X is pre-transposed per core to
Xt [F, BPC] and converted to fp16 (tolerance is 2e-2; fp16 end-to-end
error here is ~3e-4).  This removes the on-chip transpose entirely and
halves HBM traffic - the previous version spent 2/3 of its tensor-engine
flops and half its DMA bytes on transposing fp32 X on chip.

Per-core kernel: X.T streams in as 8 tiles of 1 MB (each 4 contraction
chunks, one contiguous 8 KB/partition DMA) alternating over the two
HWDGE queues (sync + scalar) - few, large DMAs sidestep the per-queue
in-flight ring stalls that capped the 32-DMA version; the PE
accumulates out.T[64, 1024] = sum_k W_k.T @ Xt_k into two PSUM banks
(fp16 operands, 1 cycle/row); epilogue split across Scalar (fused
Lrelu ACTIVATE) and Vector, output halves DMA'd on both queues.
"""

import os
import sys

import numpy as np

if "/opt/trn_rl_repo" not in sys.path:
    sys.path.insert(0, "/opt/trn_rl_repo")

B, T, HW, K = 8192, 64, 8, 64
NCORES = 8
BPC = B // NCORES  # 1024 batch rows per core
F = T * HW * HW  # 4096 contracted features
NEG_SLOPE = 0.001
FILT_LEN = 8
NKC = F // 128  # 32 contraction chunks
# Chunks per DMA tile: small head tiles so the first matmuls' inputs
# complete early (single-DMA latency is high), big middle tiles to
# keep the instruction count down, small tail tiles so the PE lag
# after the last DMA completion is short.
TILE_SIZES = [1, 1, 2, 2, 2, 4, 4, 4, 4, 4, 2, 1, 1]
assert sum(TILE_SIZES) == NKC
NT = len(TILE_SIZES)
TILE_OFFS = [sum(TILE_SIZES[:t]) for t in range(NT)]
WA_CHUNKS = 4  # chunks covered by the small first weight slice
HB = 512  # batch columns per PSUM accumulator (half of BPC)
NWARM = 8  # PE warmup matmuls to ramp the clock before real work

DB4_LO = np.array(
    [-0.010597401784997278, 0.032883011666982945, 0.030841381835986965,
     -0.18703481171888114, -0.02798376941698385, 0.6308807679295904,
     0.7148465705525415, 0.23037781330885523], dtype=np.float64)
DB4_HI = np.array(
    [-0.23037781330885523, 0.7148465705525415, -0.6308807679295904,
     -0.02798376941698385, 0.18703481171888114, 0.030841381835986965,
     0.032883011666982945, -0.010597401784997278], dtype=np.float64)


def _afb1d(x):
    # numpy mirror of the reference: reflect pad, correlate with reversed
    # filters, stride 2.  x: [N, n] float64.
    n = x.shape[-1]
    out = (n + FILT_LEN - 1) // 2
    p = 2 * (out - 1) - n + FILT_LEN
    xp = np.pad(x, ((0, 0), (p // 2, (p + 1) // 2)), mode="reflect")
    idx = 2 * np.arange(out)[:, None] + np.arange(FILT_LEN)[None, :]
    win = xp[:, idx]  # [N, out, 8]
    return win @ DB4_LO[::-1], win @ DB4_HI[::-1]


def _dwt_matrix():
    # M [64, 84] with coeffs = sig @ M (image of the identity basis).
    lo, his = np.eye(T, dtype=np.float64), []
    for _ in range(3):
        lo, hi = _afb1d(lo)
        his.append(hi)
    return np.concatenate([lo] + his, axis=-1)


def _build_bass(zero_bias=True):
    import concourse.bacc as bacc
    import concourse.mybir as mybir
    import concourse.tile as tile

    f32 = mybir.dt.float32
    f16 = mybir.dt.float16
    Lrelu = mybir.ActivationFunctionType.Lrelu
    Alu = mybir.AluOpType

    nc = bacc.Bacc("TRN2", target_bir_lowering=False, debug=False)
    x_ds = [
        nc.dram_tensor(f"x{t}", [128, s * BPC], f16, kind="ExternalInput").ap()
        for t, s in enumerate(TILE_SIZES)
    ]
    wa_d = nc.dram_tensor("wa", [128, WA_CHUNKS * K], f16,
                          kind="ExternalInput").ap()
    wb_d = nc.dram_tensor("wb", [128, (NKC - WA_CHUNKS) * K], f16,
                          kind="ExternalInput").ap()
    b_d = nc.dram_tensor("b", [K, 1], f32, kind="ExternalInput").ap()
    o_d = nc.dram_tensor("out", [K, BPC], f32, kind="ExternalOutput").ap()

    with tile.TileContext(nc) as tc:
        with (
            tc.tile_pool(name="const", bufs=1) as constp,
            tc.tile_pool(name="xt", bufs=1) as xpool,
            tc.tile_pool(name="outs", bufs=1) as outp,
            tc.tile_pool(name="acc", bufs=1, space="PSUM") as accp,
        ):
            # Weights split: tiny head slice (chunks 0..3) lands first so
            # the earliest matmuls are gated only by the first x tile.
            wsa = constp.tile([128, WA_CHUNKS * K], f16)
            nc.sync.dma_start(wsa[:], wa_d[:])
            wsb = constp.tile([128, (NKC - WA_CHUNKS) * K], f16)
            nc.scalar.dma_start(wsb[:], wb_d[:])
            if not zero_bias:
                bias = constp.tile([K, 1], f32)
                nc.scalar.dma_start(bias[:], b_d[:])

            xts = []
            for t, s in enumerate(TILE_SIZES):
                ti = xpool.tile([128, s * BPC], f16, name=f"x{t}")
                eng = nc.sync if t % 2 == 0 else nc.scalar
                eng.dma_start(ti[:], x_ds[t][:])
                xts.append(ti)

            # Warmup matmuls on a memset scratch tile: the PE clock ramps
            # with continuous busy time (0.65 -> 1.2 -> 2.4 GHz after
            # ~3us), so burn the DMA-latency window ramping up.
            warm = constp.tile([128, HB], f16)
            nc.vector.memset(warm[:], 0.0)
            wacc = accp.tile([K, HB], f32)
            for _ in range(NWARM):
                nc.tensor.matmul(wacc[:], warm[:, 0:K], warm[:],
                                 start=True, stop=True)

            acc0 = accp.tile([K, HB], f32)
            acc1 = accp.tile([K, HB], f32)
            k = 0
            for t, s in enumerate(TILE_SIZES):
                for c in range(s):
                    if k < WA_CHUNKS:
                        w_k = wsa[:, k * K:(k + 1) * K]
                    else:
                        w_k = wsb[:, (k - WA_CHUNKS) * K:
                                  (k - WA_CHUNKS + 1) * K]
                    xk = xts[t][:, c * BPC:(c + 1) * BPC]
                    nc.tensor.matmul(
                        acc0[:], w_k, xk[:, 0:HB],
                        start=(k == 0), stop=(k == NKC - 1))
                    nc.tensor.matmul(
                        acc1[:], w_k, xk[:, HB:BPC],
                        start=(k == 0), stop=(k == NKC - 1))
                    k += 1

            # Epilogue split across engines: Scalar does a fused
            # bias+LeakyReLU ACTIVATE on bank 0 while Vector handles
            # bank 1; the output halves go out on both HWDGE queues.
            o0 = outp.tile([K, HB], f32)
            nc.scalar.activation(o0[:], acc0[:], Lrelu,
                                 bias=0.0 if zero_bias else bias[:],
                                 alpha=NEG_SLOPE)
            nc.sync.dma_start(o_d[:, 0:HB], o0[:])
            o1 = outp.tile([K, HB], f32)
            if zero_bias:
                # max(acc1, slope*acc1) with only one PSUM operand per
                # instruction (hardware restriction).
                tmp = outp.tile([K, HB], f32)
                nc.vector.tensor_scalar_mul(tmp[:], acc1[:], NEG_SLOPE)
                nc.vector.scalar_tensor_tensor(
                    o1[:], acc1[:], 1.0, tmp[:],
                    op0=Alu.mult, op1=Alu.max)
            else:
                t1 = outp.tile([K, HB], f32)
                nc.vector.tensor_scalar_add(t1[:], acc1[:], bias[:])
                nc.vector.scalar_tensor_tensor(
                    o1[:], t1[:], NEG_SLOPE, t1[:],
                    op0=Alu.mult, op1=Alu.max)
            nc.scalar.dma_start(o_d[:, HB:BPC], o1[:])
    nc.compile()
    return nc


def _prep_inputs(x, conv_w, conv_b):
    M = _dwt_matrix()  # [64, 84]
    # W_eff[(t,h,w), k] = sum_c M[t,c] conv_w[k,c,h,w]
    w_eff = np.einsum("tc,kchw->thwk", M, conv_w.astype(np.float64))
    w2 = w_eff.reshape(F, K)
    # SBUF layout: wprep[p, k*K + n] = w2[k*128 + p, n]
    wprep = np.ascontiguousarray(
        w2.reshape(NKC, 128, K).transpose(1, 0, 2).reshape(128, -1)
    ).astype(np.float16)
    bias = np.ascontiguousarray(
        np.asarray(conv_b, dtype=np.float32).reshape(K, 1))
    # Per-core transposed X in fp16: chunk-major [c, k, p, j] with
    # xck[c, k, p, j] = X[c*BPC + j, k*128 + p]; each DMA tile t is the
    # contiguous slab of TILE_SIZES[t] chunks laid out [p, chunk, j].
    xf = np.asarray(x, dtype=np.float32).reshape(NCORES, BPC, NKC, 128)
    xck = np.ascontiguousarray(
        xf.astype(np.float16).transpose(0, 2, 3, 1))  # [c, k, p, j]
    xtiles = []  # [tile][core] -> [128, s*BPC] contiguous
    for t, s in enumerate(TILE_SIZES):
        o = TILE_OFFS[t]
        blk = xck[:, o:o + s]  # [c, s, 128, j]
        xtiles.append(np.ascontiguousarray(
            blk.transpose(0, 2, 1, 3)).reshape(NCORES, 128, s * BPC))
    return xtiles, wprep, bias


def kernel(x, conv_w, conv_b):
    from concourse.bass_utils import run_bass_kernel_spmd

    xtiles, wprep, bias = _prep_inputs(x, conv_w, conv_b)
    nc = _build_bass(zero_bias=not np.any(conv_b))
    wa = np.ascontiguousarray(wprep[:, :WA_CHUNKS * K])
    wb = np.ascontiguousarray(wprep[:, WA_CHUNKS * K:])
    in_maps = [
        {
            **{f"x{t}": xtiles[t][c] for t in range(NT)},
            "wa": wa, "wb": wb, "b": bias,
        }
        for c in range(NCORES)
    ]
    res = run_bass_kernel_spmd(nc, in_maps, list(range(NCORES)))
    out = np.concatenate([r["out"].T for r in res.results], axis=0)
    return np.ascontiguousarray(out, dtype=np.float32)


# revision 19
# speedup vs baseline: 1.0295x; 1.0295x over previous
"""Trainium2 kernel for nn_DWT_Features.

The reference applies a 3-level db4 DWT along the time axis of every
(batch, pixel) signal, then contracts the coefficients with a full-volume
conv kernel and applies LeakyReLU.  The DWT is a linear map sig[64] ->
coeffs[84], so the whole network collapses to a single GEMM:

    out = leaky_relu(X @ W_eff + b),  X: [B, 4096], W_eff: [4096, 64]

where W_eff[(t,h,w), k] = sum_c M[t, c] * conv_w[k, c, h, w] and M is the
64x84 DWT matrix (computed here in numpy, folded on host - O(22M) flops).

Sharding: pure data parallel, batch split across 8 cores (1024 rows each).

Host prep (sharding/relayout): X is pre-transposed per core to
Xt [F, BPC] and converted to fp16 (tolerance is 2e-2; fp16 end-to-end
error here is ~3e-4).  This removes the on-chip transpose entirely and
halves HBM traffic - the previous version spent 2/3 of its tensor-engine
flops and half its DMA bytes on transposing fp32 X on chip.

Per-core kernel: X.T streams in as 8 tiles of 1 MB (each 4 contraction
chunks, one contiguous 8 KB/partition DMA) alternating over the two
HWDGE queues (sync + scalar) - few, large DMAs sidestep the per-queue
in-flight ring stalls that capped the 32-DMA version; the PE
accumulates out.T[64, 1024] = sum_k W_k.T @ Xt_k into two PSUM banks
(fp16 operands, 1 cycle/row); epilogue split across Scalar (fused
Lrelu ACTIVATE) and Vector, output halves DMA'd on both queues.
"""

import os
import sys

import numpy as np

if "/opt/trn_rl_repo" not in sys.path:
    sys.path.insert(0, "/opt/trn_rl_repo")

B, T, HW, K = 8192, 64, 8, 64
NCORES = 8
BPC = B // NCORES  # 1024 batch rows per core
F = T * HW * HW  # 4096 contracted features
NEG_SLOPE = 0.001
FILT_LEN = 8
NKC = F // 128  # 32 contraction chunks
# Chunks per DMA tile: uniform 512 KB tiles keep many transfers in
# flight (aggregate HBM bandwidth needs concurrency); 1-chunk tail
# tiles so the PE lag after the last DMA completion is short.
TILE_SIZES = [2] * 15 + [1, 1]
assert sum(TILE_SIZES) == NKC
NT = len(TILE_SIZES)
TILE_OFFS = [sum(TILE_SIZES[:t]) for t in range(NT)]
WA_CHUNKS = 4  # chunks covered by the small first weight slice
HB = 512  # batch columns per PSUM accumulator (half of BPC)
# PE warmup matmuls: the PE clock ramps with continuous busy time
# (0.65 -> 1.2 -> 2.4 GHz after ~3us) and an idle gap can drop it
# back.  Real matmul work is ~13.8us at full clock while the x stream
# takes ~22us to land, so burn ~7us warming up; the real matmuls then
# chase the tail of the stream at 2.4 GHz without stalls.
NWARM = 26

DB4_LO = np.array(
    [-0.010597401784997278, 0.032883011666982945, 0.030841381835986965,
     -0.18703481171888114, -0.02798376941698385, 0.6308807679295904,
     0.7148465705525415, 0.23037781330885523], dtype=np.float64)
DB4_HI = np.array(
    [-0.23037781330885523, 0.7148465705525415, -0.6308807679295904,
     -0.02798376941698385, 0.18703481171888114, 0.030841381835986965,
     0.032883011666982945, -0.010597401784997278], dtype=np.float64)


def _afb1d(x):
    # numpy mirror of the reference: reflect pad, correlate with reversed
    # filters, stride 2.  x: [N, n] float64.
    n = x.shape[-1]
    out = (n + FILT_LEN - 1) // 2
    p = 2 * (out - 1) - n + FILT_LEN
    xp = np.pad(x, ((0, 0), (p // 2, (p + 1) // 2)), mode="reflect")
    idx = 2 * np.arange(out)[:, None] + np.arange(FILT_LEN)[None, :]
    win = xp[:, idx]  # [N, out, 8]
    return win @ DB4_LO[::-1], win @ DB4_HI[::-1]


def _dwt_matrix():
    # M [64, 84] with coeffs = sig @ M (image of the identity basis).
    lo, his = np.eye(T, dtype=np.float64), []
    for _ in range(3):
        lo, hi = _afb1d(lo)
        his.append(hi)
    return np.concatenate([lo] + his, axis=-1)


def _build_bass(zero_bias=True):
    import concourse.bacc as bacc
    import concourse.mybir as mybir
    import concourse.tile as tile

    f32 = mybir.dt.float32
    f16 = mybir.dt.float16
    Lrelu = mybir.ActivationFunctionType.Lrelu
    Alu = mybir.AluOpType

    nc = bacc.Bacc("TRN2", target_bir_lowering=False, debug=False)
    x_ds = [
        nc.dram_tensor(f"x{t}", [128, s * BPC], f16, kind="ExternalInput").ap()
        for t, s in enumerate(TILE_SIZES)
    ]
    wa_d = nc.dram_tensor("wa", [128, WA_CHUNKS * K], f16,
                          kind="ExternalInput").ap()
    wb_d = nc.dram_tensor("wb", [128, (NKC - WA_CHUNKS) * K], f16,
                          kind="ExternalInput").ap()
    b_d = nc.dram_tensor("b", [K, 1], f32, kind="ExternalInput").ap()
    o_d = nc.dram_tensor("out", [K, BPC], f16, kind="ExternalOutput").ap()

    with tile.TileContext(nc) as tc:
        with (
            tc.tile_pool(name="const", bufs=1) as constp,
            tc.tile_pool(name="xt", bufs=1) as xpool,
            tc.tile_pool(name="outs", bufs=1) as outp,
            tc.tile_pool(name="acc", bufs=1, space="PSUM") as accp,
        ):
            # Weights split: tiny head slice (chunks 0..3) lands first so
            # the earliest matmuls are gated only by the first x tile.
            wsa = constp.tile([128, WA_CHUNKS * K], f16)
            nc.sync.dma_start(wsa[:], wa_d[:])
            wsb = constp.tile([128, (NKC - WA_CHUNKS) * K], f16)
            nc.scalar.dma_start(wsb[:], wb_d[:])
            if not zero_bias:
                bias = constp.tile([K, 1], f32)
                nc.scalar.dma_start(bias[:], b_d[:])

            xts = []
            for t, s in enumerate(TILE_SIZES):
                ti = xpool.tile([128, s * BPC], f16, name=f"x{t}")
                eng = nc.sync if t % 2 == 0 else nc.scalar
                eng.dma_start(ti[:], x_ds[t][:])
                xts.append(ti)

            # Warmup matmuls on a memset scratch tile: the PE clock ramps
            # with continuous busy time (0.65 -> 1.2 -> 2.4 GHz after
            # ~3us), so burn the DMA-latency window ramping up.
            warm = constp.tile([128, HB], f16)
            nc.vector.memset(warm[:], 0.0)
            wacc = accp.tile([K, HB], f32)
            for _ in range(NWARM):
                nc.tensor.matmul(wacc[:], warm[:, 0:K], warm[:],
                                 start=True, stop=True)

            acc0 = accp.tile([K, HB], f32)
            acc1 = accp.tile([K, HB], f32)
            k = 0
            for t, s in enumerate(TILE_SIZES):
                for c in range(s):
                    if k < WA_CHUNKS:
                        w_k = wsa[:, k * K:(k + 1) * K]
                    else:
                        w_k = wsb[:, (k - WA_CHUNKS) * K:
                                  (k - WA_CHUNKS + 1) * K]
                    xk = xts[t][:, c * BPC:(c + 1) * BPC]
                    nc.tensor.matmul(
                        acc0[:], w_k, xk[:, 0:HB],
                        start=(k == 0), stop=(k == NKC - 1))
                    nc.tensor.matmul(
                        acc1[:], w_k, xk[:, HB:BPC],
                        start=(k == 0), stop=(k == NKC - 1))
                    k += 1

            # Epilogue split across engines: Scalar does a fused
            # bias+LeakyReLU ACTIVATE on bank 0 while Vector handles
            # bank 1; fp16 output halves go out on both HWDGE queues.
            o0 = outp.tile([K, HB], f16)
            nc.scalar.activation(o0[:], acc0[:], Lrelu,
                                 bias=0.0 if zero_bias else bias[:],
                                 alpha=NEG_SLOPE)
            nc.sync.dma_start(o_d[:, 0:HB], o0[:])
            o1 = outp.tile([K, HB], f16)
            if zero_bias:
                # max(acc1, slope*acc1) with only one PSUM operand per
                # instruction (hardware restriction).
                tmp = outp.tile([K, HB], f32)
                nc.vector.tensor_scalar_mul(tmp[:], acc1[:], NEG_SLOPE)
                nc.vector.scalar_tensor_tensor(
                    o1[:], acc1[:], 1.0, tmp[:],
                    op0=Alu.mult, op1=Alu.max)
            else:
                t1 = outp.tile([K, HB], f32)
                nc.vector.tensor_scalar_add(t1[:], acc1[:], bias[:])
                nc.vector.scalar_tensor_tensor(
                    o1[:], t1[:], NEG_SLOPE, t1[:],
                    op0=Alu.mult, op1=Alu.max)
            nc.scalar.dma_start(o_d[:, HB:BPC], o1[:])
    nc.compile()
    return nc


def _prep_inputs(x, conv_w, conv_b):
    M = _dwt_matrix()  # [64, 84]
    # W_eff[(t,h,w), k] = sum_c M[t,c] conv_w[k,c,h,w]
    w_eff = np.einsum("tc,kchw->thwk", M, conv_w.astype(np.float64))
    w2 = w_eff.reshape(F, K)
    # SBUF layout: wprep[p, k*K + n] = w2[k*128 + p, n]
    wprep = np.ascontiguousarray(
        w2.reshape(NKC, 128, K).transpose(1, 0, 2).reshape(128, -1)
    ).astype(np.float16)
    bias = np.ascontiguousarray(
        np.asarray(conv_b, dtype=np.float32).reshape(K, 1))
    # Per-core transposed X in fp16: chunk-major [c, k, p, j] with
    # xck[c, k, p, j] = X[c*BPC + j, k*128 + p]; each DMA tile t is the
    # contiguous slab of TILE_SIZES[t] chunks laid out [p, chunk, j].
    xf = np.asarray(x, dtype=np.float32).reshape(NCORES, BPC, NKC, 128)
    xck = np.ascontiguousarray(
        xf.astype(np.float16).transpose(0, 2, 3, 1))  # [c, k, p, j]
    xtiles = []  # [tile][core] -> [128, s*BPC] contiguous
    for t, s in enumerate(TILE_SIZES):
        o = TILE_OFFS[t]
        blk = xck[:, o:o + s]  # [c, s, 128, j]
        xtiles.append(np.ascontiguousarray(
            blk.transpose(0, 2, 1, 3)).reshape(NCORES, 128, s * BPC))
    return xtiles, wprep, bias


def kernel(x, conv_w, conv_b):
    from concourse.bass_utils import run_bass_kernel_spmd

    xtiles, wprep, bias = _prep_inputs(x, conv_w, conv_b)
    nc = _build_bass(zero_bias=not np.any(conv_b))
    wa = np.ascontiguousarray(wprep[:, :WA_CHUNKS * K])
    wb = np.ascontiguousarray(wprep[:, WA_CHUNKS * K:])
    in_maps = [
        {
            **{f"x{t}": xtiles[t][c] for t in range(NT)},
            "wa": wa, "wb": wb, "b": bias,
        }
        for c in range(NCORES)
    ]
    res = run_bass_kernel_spmd(nc, in_maps, list(range(NCORES)))
    out = np.concatenate(
        [r["out"].astype(np.float32).T for r in res.results], axis=0)
    return np.ascontiguousarray(out, dtype=np.float32)
